# revision 44
# baseline (speedup 1.0000x reference)
"""Trainium2 Bass kernel for nn_Attention_40570261078258.

Computes, for x:(8,128,64,64), Wq/Wk/Wv:(128,128), bq/bk/bv:(128,):
    xf = x.reshape(N, C, L);  L = 4096
    q/k/v = W @ xf + b                  -> (N, L, C) logical
    scores = q @ k^T / sqrt(C)          -> (N, L, L)
    attn = softmax(scores, axis=0)      # over the BATCH axis (torch legacy dim=0)
    out = attn @ v                      -> (N, L, C)
    return x + out.reshape(N, C, H, W)  # reinterpreting (L,C) memory as (C,H,W)

Sharding: 2 l-halves x 4 m-quarters across the 8 cores. The batch-axis
softmax couples all 8 n at each (l, m), so every core keeps all batch
elements for its (l, m) block; splitting the (l, m) plane 8 ways also cuts
the replicated k/v projection work 4x vs l-only sharding. Each core emits an
UNSUMMED partial over its m-quarter; the host adds the 4 partials per l-half
and the residual - pure glue next to the O(N L^2 C) attention.

Per core (all matmuls bf16; the host pre-casts x and the weights to bf16 and
packs weights/biases into two tensors so phase 1 starts with 2 const DMAs):

phase 1 - projections, ordered for the phase-2 critical path [k all-n,
  q-low-half, vT, q-high-half], x streamed in 2-batch DMA chunks alternating
  between the SP and Act HWDGE queues. PSUM evictions (the phase-1
  bottleneck; GPSIMD cannot touch PSUM on trn2) are spread greedily over
  Act (activation+bias) and DVE (scalar_tensor_tensor+bias); half the vT
  evictions run two-stage (Act plain copy, then Pool adds bv in SBUF).

phase 2 - 64 jobs of (8n x 128m x 256l), software-pipelined with SKEW jobs
  of scores/exp lookahead:
    scores   PE, 8 matmuls into a (128,1024)x2 PSUM rotation [4 banks]
    exp      Act, 2x 1024-wide Exp evictions (the 2076ns/job pace-setter;
             exp is Act-only and its 109us is the hard floor of this graph)
    n-sum    DVE, 3-level pairwise bf16 tree (2x DVE mode)
    recip    DVE (Act's Reciprocal is banned for accuracy)
    A=E*r    DVE 3D-broadcast tensor_mul for 5 batch groups (2x mode),
             Pool tensor_mul for 3 (GPSIMD 0.42 efficiency - Pool is weak)
    AV       PE, 8 matmuls accumulating over the 8 m-tiles into a
             (128,1024)x2 PSUM accumulator pair [4 banks]
    out      per l-tile: Act+DVE evict the accumulators to bf16 SBUF,
             one 3D DMA writes the (lt, 8n, c, l) partial to DRAM.

The schedule never round-trips a value chain across engines mid-job (each
in-order engine stream would serialize on the other), and keeps PE fed
continuously where possible - the cost model's p-state ramp makes cold
matmuls up to 3.7x slower.
"""

import math

import numpy as np

import concourse.bacc as bacc
import concourse.bass as bass
import concourse.mybir as mybir
import concourse.tile as tile
from concourse.bass_utils import run_bass_kernel_spmd

N, C, H, W = 8, 128, 64, 64
L = H * W            # 4096 pixels
NCORES = 8
NLH = 2              # l-halves (cores axis 0)
NMQ = 4              # m-quarters (cores axis 1)
LQ = L // NLH        # 2048 query positions per core
MQ = L // NMQ        # 1024 key positions per core
LT = 256             # l per job
NLT = LQ // LT       # 8 l-tiles
NMT = MQ // 128      # 8 m-tiles of 128

FP = mybir.dt.float32
BF = mybir.dt.bfloat16
AF = mybir.ActivationFunctionType
ALU = mybir.AluOpType

SKEW = 9
RAMP = 0
EV_SEED_A = 0.0
EV_COST_A = 1.4
Z_ON_POOL = False
OB_ON_ACT = 1
DEBUG = False
MUL_SPLIT_W = 0
AV_FIRST = False
PJA_BUFS = 2
PJB_BUFS = 2
TAIL_LT = 7
SPLIT_OUT_DMA = True
TAIL_DVE_GROUPS = 5
LAST_MT = 8
LAST_DVE_GROUPS = 6
E_BUFS = 12          # E-tile slots
A_BUFS = 8           # attn-tile slots
DIV_DVE_GROUPS = 4   # of the 8 normalize-divide batch groups, how many on DVE

# Set by test harness to capture a profile.
TRACE = False
LAST_RESULTS = None


def build():
    nc = bacc.Bacc(
        "TRN2",
        target_bir_lowering=False,
        debug=False,
        enable_asserts=True,
        num_devices=NCORES,
    )

    # Host pre-casts x to bf16 and pre-transposes weights (wt[c,o] = W[o,c]).
    xq = nc.dram_tensor("xq", [N, C, LQ], BF, kind="ExternalInput").ap()
    xk = nc.dram_tensor("xk", [N, C, MQ], BF, kind="ExternalInput").ap()
    wpack = nc.dram_tensor("wpack", [C, 3 * C], BF, kind="ExternalInput").ap()
    bpack = nc.dram_tensor("bpack", [128, C + 2], FP, kind="ExternalInput").ap()
    # Partial attention output for this core's (l-half, m-quarter):
    # out[lt, n, c, l] in bf16; host sums the 4 m-quarter partials.
    out = nc.dram_tensor("out", [NLT, N, C, LT], BF, kind="ExternalOutput").ap()
    dbg = None
    if DEBUG:
        dbg = {
            "dq": nc.dram_tensor("dq", [N, C, LQ], BF, kind="ExternalOutput").ap(),
            "dk": nc.dram_tensor("dk", [N, C, MQ], BF, kind="ExternalOutput").ap(),
            "dv": nc.dram_tensor("dv", [N, 128, NMT * C], BF, kind="ExternalOutput").ap(),
            "de": nc.dram_tensor("de", [128, 2048], BF, kind="ExternalOutput").ap(),
            "dz": nc.dram_tensor("dz", [128, LT], BF, kind="ExternalOutput").ap(),
            "da": nc.dram_tensor("da", [128, 2048], BF, kind="ExternalOutput").ap(),
        }

    with tile.TileContext(nc) as tc:
        _emit(nc, tc, xq, xk, wpack, bpack, out, dbg)

    nc.compile()
    return nc


def _emit(nc, tc, xq, xk, wpack, bpack, out, dbg=None):
    from contextlib import ExitStack

    with ExitStack() as ctx:
        cpool = ctx.enter_context(tc.tile_pool(name="const", bufs=1))
        resid = ctx.enter_context(tc.tile_pool(name="resident", bufs=1))

        # --- constants (two packed DMAs: weights bf16, biases f32) --------
        w_t = cpool.tile([C, 3 * C], BF, tag="wpk")
        nc.sync.dma_start(w_t[:], wpack)
        b_t = cpool.tile([128, C + 2], FP, tag="bpk")
        nc.sync.dma_start(b_t[:], bpack)
        wT = {
            "q": w_t[:, 0:C],
            "k": w_t[:, C : 2 * C],
            "v": w_t[:, 2 * C : 3 * C],
        }
        bv_rep = b_t[:, 0:C]
        bq_t = b_t[:, C : C + 1]
        bk_t = b_t[:, C + 1 : C + 2]

        # --- resident activations -----------------------------------------
        # q_sb[n]: (c, l) this core's l-half;  k_sb[n]: (c, m) m-quarter;
        # vT_sb[n]: 8 chunks of (m128, c) side by side; all bf16.
        q_sb = [resid.tile([C, LQ], BF, tag=f"q{n}", name=f"q_sb{n}") for n in range(N)]
        k_sb = [resid.tile([C, MQ], BF, tag=f"k{n}", name=f"k_sb{n}") for n in range(N)]
        vT_sb = [resid.tile([128, NMT * C], BF, tag=f"v{n}", name=f"vT_sb{n}") for n in range(N)]

        # --- phase 1: projections ------------------------------------------
        # One xk DMA per n feeds both the k and vT projections. q arrives as
        # two half DMAs per n so eviction work can spread. Evictions are
        # balanced across Act (k + some q), DVE (most q, some vT) and Pool
        # (most vT, some q).
        # Preload the Exp activation table early (overlaps phase-1 DMAs).
        warm = cpool.tile([128, 16], BF, tag="warm")
        nc.vector.memset(warm[:], 0.0)
        nc.scalar.activation(warm[:], warm[:], AF.Exp)

        def stt(eng, dst, src, bias_bc):
            eng.scalar_tensor_tensor(dst, src, 1.0, bias_bc, ALU.mult, ALU.add)

        with (
            tc.tile_pool(name="xkin", bufs=1) as xk_pool,
            tc.tile_pool(name="xqin", bufs=1) as xq_pool,
            tc.tile_pool(name="pja", bufs=PJA_BUFS, space="PSUM") as pja_psum,
            tc.tile_pool(name="pjb", bufs=PJB_BUFS, space="PSUM") as pjb_psum,
        ):
            bq_bc = bq_t.broadcast_to((128, 1024))
            bk_bc = bk_t.broadcast_to((128, 1024))
            # Batched input DMAs, interleaved across the SP and Act issue
            # queues so the exclusive DMA device streams x continuously.
            # PE order: k(n0-3), q0(n0-3), k(n4-7), q0(n4-7), vT(all), q1(all)
            # - vT (needs only xk) fills the PE gap while xq1 transfers.
            # Evictions are assigned greedily to the least-loaded engine
            # (Act/DVE/Pool for per-partition-bias evicts, DVE/Pool for vT).
            xk_all = xk_pool.tile([C, N * MQ], BF, tag="xka")
            xq0 = xq_pool.tile([C, N * 1024], BF, tag="xq0", name="xq0_all")
            xq1 = xq_pool.tile([C, N * 1024], BF, tag="xq1", name="xq1_all")
            def dma_xk(n0, n1):
                nc.sync.dma_start(
                    xk_all[:, n0 * MQ : n1 * MQ].rearrange(
                        "c (n m) -> c n m", n=n1 - n0
                    ),
                    xk[n0:n1].rearrange("n c m -> c n m"),
                )

            def dma_xq(dst, h, l0):
                nc.sync.dma_start(
                    dst[:, h * 2048 : (h + 1) * 2048].rearrange(
                        "c (n l) -> c n l", n=2
                    ),
                    xq[2 * h : 2 * h + 2, :, l0 : l0 + 1024].rearrange(
                        "n c l -> c n l"
                    ),
                )

            # All input DMAs issue from the otherwise-idle SP queue (issuing
            # from Act would hold the Act sequencer ~1us per DMA and delay
            # its phase-1 evictions). Arrival order matches consumption.
            for n0, n1 in [(0, 1), (1, 2), (2, 4)]:
                dma_xk(n0, n1)
            dma_xq(xq0, 0, 0)
            for n0, n1 in [(4, 6), (6, 8)]:
                dma_xk(n0, n1)
            for h in range(1, 4):
                dma_xq(xq0, h, 0)
            for h in range(4):
                dma_xq(xq1, h, 1024)

            # greedy eviction lane assignment by accumulated engine time
            load = {"A": EV_SEED_A, "D": 0.0, "P": 0.0}
            cost = {"A": EV_COST_A, "D": 1.19, "P": 1.52}

            def evict(dst, psrc, bias_bc, lanes="AD"):
                lane = min(lanes, key=lambda e: load[e] + cost[e])
                load[lane] += cost[lane]
                if lane == "D":
                    stt(nc.vector, dst, psrc, bias_bc)
                elif lane == "P":
                    stt(nc.gpsimd, dst, psrc, bias_bc)
                else:
                    nc.scalar.activation(
                        dst, psrc, AF.Identity, bias=bias_bc[:, 0:1]
                    )

            def kproj(n):
                pk = pja_psum.tile([128, 1024], FP, tag="pja")
                for half in range(2):
                    nc.tensor.matmul(
                        pk[:, half * 512 : (half + 1) * 512],
                        wT["k"],
                        xk_all[:, n * MQ + half * 512 : n * MQ + (half + 1) * 512],
                        start=True,
                        stop=True,
                    )
                evict(k_sb[n][:], pk[:], bk_bc)

            def qproj(n, hb, xsrc):
                pq = (pja_psum if hb == 0 else pjb_psum).tile(
                    [128, 1024], FP, tag="pja" if hb == 0 else "pjb"
                )
                for half in range(2):
                    nc.tensor.matmul(
                        pq[:, half * 512 : (half + 1) * 512],
                        wT["q"],
                        xsrc[:, n * 1024 + half * 512 : n * 1024 + (half + 1) * 512],
                        start=True,
                        stop=True,
                    )
                evict(q_sb[n][:, hb * 1024 : (hb + 1) * 1024], pq[:], bq_bc)

            vbias = bv_rep.unsqueeze(1).broadcast_to((128, NMT, C))

            def vproj(n):
                pv = pjb_psum.tile([128, 1024], FP, tag="pjb")
                for ch in range(NMT):
                    # (m128, c) = x_chunk^T @ WvT ; full rate at bf16.
                    nc.tensor.matmul(
                        pv[:, ch * C : (ch + 1) * C],
                        xk_all[:, n * MQ + ch * 128 : n * MQ + (ch + 1) * 128],
                        wT["v"],
                        start=(ch % 4 == 0),
                        stop=(ch % 4 == 3),
                    )
                vdst = vT_sb[n][:].rearrange("p (s c) -> p s c", s=NMT)
                vsrc = pv[:].rearrange("p (s c) -> p s c", s=NMT)
                if n % 2 == 0:
                    evict(vdst, vsrc, vbias, lanes="D")
                else:
                    # two-stage: Act copies psum->sbuf (no bias; Act bias is
                    # per-partition only), Pool adds bv in SBUF off the
                    # critical path (GPSIMD may not touch PSUM, SBUF is fine).
                    nc.scalar.copy(vT_sb[n][:], pv[:])
                    nc.gpsimd.tensor_add(vdst, vdst, vbias)
                    load["A"] += 0.87

            for g in range(2):
                for n in range(4 * g, 4 * g + 4):
                    kproj(n)
                for n in range(4 * g, 4 * g + 4):
                    qproj(n, 0, xq0)
            for n in range(N):
                vproj(n)
            for n in range(N):
                qproj(n, 1, xq1)
            if dbg is not None:
                for n in range(N):
                    nc.sync.dma_start(dbg["dq"][n], q_sb[n][:])
                    nc.sync.dma_start(dbg["dk"][n], k_sb[n][:])
                    nc.sync.dma_start(dbg["dv"][n], vT_sb[n][:])

        # --- phase 2: attention with softmax over batch --------------------
        inv_sqrt_c = 1.0 / math.sqrt(C)
        with (
            tc.tile_pool(name="scp", bufs=2, space="PSUM") as sc_psum,
            tc.tile_pool(name="avp", bufs=2, space="PSUM") as av_psum,
            tc.tile_pool(name="soft", bufs=1) as soft_pool,
            tc.tile_pool(name="ost", bufs=1) as ost_pool,
        ):
            pend = {}   # (lt, mt) -> E tile (128, 8n x 256l)
            avps = {}   # lt -> [avp_j0, avp_j1]

            def emit_scores(lt, mt):
                l0 = lt * LT
                e = soft_pool.tile([128, 2048], BF, tag="E", bufs=E_BUFS)
                for j in range(2):
                    ps = sc_psum.tile([128, 1024], FP, tag="sc")
                    for i in range(4):
                        n = 4 * j + i
                        nc.tensor.matmul(
                            ps[:, i * LT : (i + 1) * LT],
                            k_sb[n][:, mt * 128 : (mt + 1) * 128],
                            q_sb[n][:, l0 : l0 + LT],
                            start=True,
                            stop=True,
                        )
                    nc.scalar.activation(
                        e[:, j * 1024 : (j + 1) * 1024],
                        ps[:],
                        AF.Exp,
                        scale=inv_sqrt_c,
                    )
                pend[(lt, mt)] = e

            def emit_soft_av(lt, mt):
                if mt == 0:
                    avps[lt] = [
                        av_psum.tile([128, 1024], FP, tag="av", name=f"avp{lt}_{j}")
                        for j in range(2)
                    ]
                avp = avps[lt]
                e = pend.pop((lt, mt))
                # n-sum tree: s1+s2 on DVE, final level on Pool (stt-add
                # lowers to TensorScalarPtr: 0.6 gpsimd efficiency).
                s1 = soft_pool.tile([128, 1024], BF, tag="zt1", bufs=3)
                nc.vector.tensor_add(s1[:], e[:, 0:1024], e[:, 1024:2048])
                s2 = soft_pool.tile([128, 512], BF, tag="zt2", bufs=3)
                nc.vector.tensor_add(s2[:], s1[:, 0:512], s1[:, 512:1024])
                z = soft_pool.tile([128, LT], BF, tag="zr", bufs=4)
                if Z_ON_POOL:
                    nc.gpsimd.tensor_add(z[:], s2[:, 0:LT], s2[:, LT : 2 * LT])
                else:
                    nc.vector.tensor_add(z[:], s2[:, 0:LT], s2[:, LT : 2 * LT])
                r = soft_pool.tile([128, LT], BF, tag="rr", bufs=4)
                with nc.allow_low_precision(
                    "softmax denom is a sum of 8 O(1..500) exps; bf16 ok"
                ):
                    nc.vector.reciprocal(r[:], z[:])
                # normalize: A[n] = E[n] * r; DVE 2x tensor_mul for the first
                # groups, Pool stt-mul for the rest.
                a = soft_pool.tile([128, 2048], BF, tag="A", bufs=A_BUFS)
                gd = DIV_DVE_GROUPS if lt < TAIL_LT else TAIL_DVE_GROUPS
                if lt == NLT - 1 and mt >= LAST_MT:
                    gd = LAST_DVE_GROUPS
                w = MUL_SPLIT_W  # cols of group gd handled by DVE (rest Pool)
                if gd:
                    nc.vector.tensor_mul(
                        a[:, : gd * LT].rearrange("p (g l) -> p g l", g=gd),
                        e[:, : gd * LT].rearrange("p (g l) -> p g l", g=gd),
                        r[:].unsqueeze(1).broadcast_to((128, gd, LT)),
                    )
                if w:
                    c0 = gd * LT
                    nc.vector.tensor_mul(
                        a[:, c0 : c0 + w], e[:, c0 : c0 + w], r[:, 0:w]
                    )
                if w < LT:
                    c0 = gd * LT
                    nc.gpsimd.tensor_mul(
                        a[:, c0 + w : c0 + LT],
                        e[:, c0 + w : c0 + LT],
                        r[:, w:LT],
                    )
                if gd < 7:
                    gp = 7 - gd
                    nc.gpsimd.tensor_mul(
                        a[:, (gd + 1) * LT :].rearrange("p (g l) -> p g l", g=gp),
                        e[:, (gd + 1) * LT :].rearrange("p (g l) -> p g l", g=gp),
                        r[:].unsqueeze(1).broadcast_to((128, gp, LT)),
                    )
                if dbg is not None and lt == 0 and mt == 0:
                    nc.sync.dma_start(dbg["de"], e[:])
                    nc.sync.dma_start(dbg["dz"], z[:])
                    nc.sync.dma_start(dbg["da"], a[:])
                for j in range(2):
                    for i in range(4):
                        n = 4 * j + i
                        sl = slice(i * LT, (i + 1) * LT)
                        nc.tensor.matmul(
                            avp[j][:, sl],
                            vT_sb[n][:, mt * C : (mt + 1) * C],
                            a[:, n * LT : (n + 1) * LT],
                            start=(mt == 0 and i % 2 == 0),
                            stop=(mt == NMT - 1 and i % 2 == 1),
                        )
                if mt == NMT - 1:
                    avp = avps.pop(lt)
                    ob = ost_pool.tile([128, 2048], BF, tag="ob", bufs=2)
                    if OB_ON_ACT == 3:
                        eng0 = eng1 = (nc.scalar if lt % 2 == 0 else nc.vector)
                    else:
                        eng0 = nc.scalar if OB_ON_ACT >= 1 else nc.vector
                        eng1 = nc.scalar if OB_ON_ACT >= 2 else nc.vector
                    for eng, sl, av_j in (
                        (eng0, slice(0, 1024), 0),
                        (eng1, slice(1024, 2048), 1),
                    ):
                        if eng is nc.scalar:
                            nc.scalar.copy(ob[:, sl], avp[av_j][:])
                        else:
                            nc.vector.tensor_copy(ob[:, sl], avp[av_j][:])
                    if SPLIT_OUT_DMA:
                        for jh in range(2):
                            nc.sync.dma_start(
                                out[lt, 4 * jh : 4 * jh + 4].rearrange(
                                    "g c l -> c g l"
                                ),
                                ob[:, 1024 * jh : 1024 * (jh + 1)].rearrange(
                                    "p (g l) -> p g l", g=4
                                ),
                            )
                    else:
                        nc.sync.dma_start(
                            out[lt].rearrange("g c l -> c g l"),
                            ob[:].rearrange("p (g l) -> p g l", g=N),
                        )

            jobs = [(lt, mt) for lt in range(NLT) for mt in range(NMT)]
            nj = len(jobs)
            done = 0
            for t, job in enumerate(jobs):
                if AV_FIRST:
                    lag = min(SKEW, max(RAMP, nj - 1 - t))
                    while done < t - lag:
                        emit_soft_av(*jobs[done])
                        done += 1
                    emit_scores(*job)
                    while done < t + 1 - lag:
                        emit_soft_av(*jobs[done])
                        done += 1
                else:
                    emit_scores(*job)
                    lag = min(SKEW, max(RAMP, nj - 1 - t))
                    while done < t + 1 - lag:
                        emit_soft_av(*jobs[done])
                        done += 1
            while done < nj:
                emit_soft_av(*jobs[done])
                done += 1


_NC = None


def _get_nc():
    global _NC
    if _NC is None:
        _NC = build()
    return _NC


def kernel(x, Wq, bq, Wk, bk, Wv, bv):
    global LAST_RESULTS
    import ml_dtypes

    bf16 = ml_dtypes.bfloat16
    x = np.ascontiguousarray(np.asarray(x, dtype=np.float32))
    xf = x.reshape(N, C, L)
    xf_bf = xf.astype(bf16)
    wpack = np.concatenate(
        [
            np.asarray(Wq, dtype=np.float32).T,
            np.asarray(Wk, dtype=np.float32).T,
            np.asarray(Wv, dtype=np.float32).T,
        ],
        axis=1,
    ).astype(bf16)
    bpack = np.concatenate(
        [
            np.broadcast_to(np.asarray(bv, dtype=np.float32).reshape(1, C), (128, C)),
            np.broadcast_to(np.asarray(bq, dtype=np.float32).reshape(C, 1), (C, 1)),
            np.broadcast_to(np.asarray(bk, dtype=np.float32).reshape(C, 1), (C, 1)),
        ],
        axis=1,
    )
    bpack = np.ascontiguousarray(bpack, dtype=np.float32)

    in_maps = []
    for d in range(NCORES):
        il, im = d // NMQ, d % NMQ
        in_maps.append(
            {
                "xq": np.ascontiguousarray(xf_bf[:, :, il * LQ : (il + 1) * LQ]),
                "xk": np.ascontiguousarray(xf_bf[:, :, im * MQ : (im + 1) * MQ]),
                "wpack": wpack,
                "bpack": bpack,
            }
        )

    nc = _get_nc()
    res = run_bass_kernel_spmd(
        nc, in_maps, core_ids=list(range(NCORES)), trace=TRACE
    )
    LAST_RESULTS = res
    # Each core returns out[lt, n, c, l] (bf16) — the attention partial for
    # its (l-half, m-quarter). Sum quarters, reorder to flat (l, c), add x.
    att = np.zeros((N, L, C), dtype=np.float32)
    for d in range(NCORES):
        il, im = d // NMQ, d % NMQ
        o = np.asarray(res.results[d]["out"]).astype(np.float32)
        # (lt, n, c, l) -> (n, lt, l, c)
        o = o.transpose(1, 0, 3, 2).reshape(N, LQ, C)
        att[:, il * LQ : (il + 1) * LQ, :] += o
    return (x.reshape(N, C * H * W) + att.reshape(N, L * C)).reshape(N, C, H, W)
